# revision 1
# baseline (speedup 1.0000x reference)
"""AttnEmbed Trainium2 kernel.

8 NeuronCores, data-parallel over the 64 (batch, spatial-tile) units; core c
handles batch c//2 and 8 of that batch's 16 spatial tiles.

Math restructuring (all exact):
  - scores = (im_s+pos) @ kw^T @ q^T is computed as imT-chunks x kq with
    kq = kw^T @ q^T ([256,64] per batch) -- k is never materialized.
  - kw_b is softmax-invariant (constant along the key axis) and dropped.
  - pos enters scores additively, so exp(s/16) = exp(s_im/16) * epos with
    epos = exp((posT x kq)/16) computed ONCE per batch.
  - vw_b folded into v before the softmax-weighted sum (weights sum to 1).
  - Adaptive-avg-pool /64 folded into resup weights on host.
  - No softmax max-subtraction (logits ~N(0, 0.2^2), exp is safe).

Layouts: feature-major for projections (contract dim on partitions);
attention weights as w^T [L-part, 64 q] so exp is full-width and the value
contraction needs no transposes. Layernorm rstd = exp(-0.5*ln(v+eps)) keeps
the attention phase inside the natural_log_exp ACT table set; gelu is
batched in its own phase (2 table switches per core total).
"""

import numpy as np
from contextlib import ExitStack

import concourse.bass as bass
import concourse.tile as tile
from concourse import bacc, mybir
from concourse.tile_rust import add_dep_helper
import concourse.bacc as _bacc_mod
import concourse.hw_specs as _hw_specs

_orig_gat = _hw_specs.get_activation_tables


def _steered_tables(arch):
    t = _orig_gat(arch)
    af = mybir.ActivationFunctionType
    for name, funcs in t.items():
        if name != "natural_log_exp_and_others":
            funcs.discard(af.Exp)
            funcs.discard(af.Ln)
    return t


_bacc_mod.get_activation_tables = _steered_tables
from concourse.bass_utils import run_bass_kernel_spmd

F32 = mybir.dt.float32
F32R = mybir.dt.float32r
BF16 = mybir.dt.bfloat16
AF = mybir.ActivationFunctionType
OP = mybir.AluOpType

B = 4
L = 4096               # tokens per spatial tile (64x64)
D = 256                # model dim
NQ = 64                # queries
FF = 1024              # ffn dim
NCORES = 8
UPC = 8                # units (s-tiles) per core
EPS = 1e-5
SCALE = 1.0 / 16.0     # 1/sqrt(D)
NLC = L // 128         # 32 L-chunks

# knobs
TRACE = False
ATTN_BF16 = True       # expw / epos / v stored bf16 (psum accum stays fp32)
IM_BF16 = True        # im tiles bf16 (halves DMA; scores/v matmuls bf16)
LAST_EXEC_NS = None
LAST_RESULTS = None


def _pos_sine_np():
    nf = D // 2
    y, x = 64, 64
    ye = np.arange(1, y + 1, dtype=np.float32)[:, None] * np.ones((1, x), np.float32)
    xe = np.arange(1, x + 1, dtype=np.float32)[None, :] * np.ones((y, 1), np.float32)
    dim_t = (10000.0 ** (2.0 * (np.arange(nf) // 2) / nf)).astype(np.float32)
    px = xe[:, :, None] / dim_t
    py = ye[:, :, None] / dim_t
    px = np.stack((np.sin(px[..., 0::2]), np.cos(px[..., 1::2])), axis=-1).reshape(y, x, nf)
    py = np.stack((np.sin(py[..., 0::2]), np.cos(py[..., 1::2])), axis=-1).reshape(y, x, nf)
    return np.concatenate([py, px], axis=-1).reshape(L, D).astype(np.float32)


def build_nc(flags):
    im_dt = BF16 if IM_BF16 else F32R
    at_dt = BF16 if ATTN_BF16 else F32R
    im_r = lambda ap: ap
    f_r = lambda ap: ap
    at_r = lambda ap: ap
    ts = bass.ts

    nc = bacc.Bacc(None, target_bir_lowering=False)
    dt_im = nc.dram_tensor("imt", [UPC, 2, 128, L], im_dt, kind="ExternalInput")
    dt_imtm = nc.dram_tensor("imtm", [UPC, 128, NLC, 258], im_dt, kind="ExternalInput")
    dt_pos = nc.dram_tensor("posT", [2, 128, L], im_dt, kind="ExternalInput")
    dt_emb = nc.dram_tensor("emb_b", [NQ, D], F32, kind="ExternalInput")
    dt_embT = nc.dram_tensor("embT", [2, 128, NQ], F32R, kind="ExternalInput")
    dt_vwT = nc.dram_tensor("vwT", [2, 128, D], F32R, kind="ExternalInput")
    dt_kww = nc.dram_tensor("kww", [2, 128, D], F32R, kind="ExternalInput")
    dt_qwT = nc.dram_tensor("qwT", [2, 128, D], F32R, kind="ExternalInput")
    dt_ewT = nc.dram_tensor("embWT", [2, 128, 768], F32R, kind="ExternalInput")
    dt_l1T = nc.dram_tensor("lin1T", [2, 128, FF], BF16, kind="ExternalInput")
    dt_l2T = nc.dram_tensor("lin2T", [8, 128, D], BF16, kind="ExternalInput")
    dt_ruT = nc.dram_tensor("resupT", [2, 128, D], BF16, kind="ExternalInput")
    dt_e64 = nc.dram_tensor("eye64", [64, 64], F32R, kind="ExternalInput")
    dt_e128 = nc.dram_tensor("eye128", [128, 128], F32R, kind="ExternalInput")
    dt_qb = nc.dram_tensor("qw_bT", [2, 128, 1], F32, kind="ExternalInput")
    dt_ebq = nc.dram_tensor("embW_bqT", [2, 128, 1], F32, kind="ExternalInput")
    dt_ebv = nc.dram_tensor("embW_bvT", [2, 128, 1], F32, kind="ExternalInput")
    dt_vwrep = nc.dram_tensor("vw_rep", [128, D], F32, kind="ExternalInput")
    dt_l1b = nc.dram_tensor("lin1b_row", [1, FF], BF16, kind="ExternalInput")
    dt_l2brep = nc.dram_tensor("lin2b_rep", [NQ, D], F32, kind="ExternalInput")
    dt_rubrep = nc.dram_tensor("resupb_rep", [NQ, D], F32, kind="ExternalInput")
    dt_n1g = nc.dram_tensor("n1g_rep", [NQ, D], F32, kind="ExternalInput")
    dt_n1b = nc.dram_tensor("n1b_rep", [NQ, D], F32, kind="ExternalInput")
    dt_png = nc.dram_tensor("png_rep", [NQ, D], F32, kind="ExternalInput")
    dt_pnb = nc.dram_tensor("pnb_rep", [NQ, D], F32, kind="ExternalInput")
    dt_ones2 = nc.dram_tensor("ones_col2", [128, 2], F32R, kind="ExternalInput")
    dt_onesrow = nc.dram_tensor("ones_rowq", [1, NQ], BF16, kind="ExternalInput")
    dt_out = nc.dram_tensor("out", [UPC, NQ, D], F32, kind="ExternalOutput")

    with tile.TileContext(nc) as tc, ExitStack() as ctx:
        pc = ctx.enter_context(tc.tile_pool(name="pc", bufs=1))
        pim = ctx.enter_context(tc.tile_pool(name="pim", bufs=4))
        pvsb = ctx.enter_context(tc.tile_pool(name="pvsb", bufs=2))
        pexp = ctx.enter_context(tc.tile_pool(name="pexp", bufs=2))
        pa1 = ctx.enter_context(tc.tile_pool(name="pa1", bufs=UPC))
        pa1t = ctx.enter_context(tc.tile_pool(name="pa1t", bufs=UPC))
        pht = ctx.enter_context(tc.tile_pool(name="pht", bufs=UPC))
        pmisc = ctx.enter_context(tc.tile_pool(name="pmisc", bufs=2))
        pnarrow = ctx.enter_context(tc.tile_pool(name="pnarrow", bufs=3))
        pp_v = ctx.enter_context(tc.tile_pool(name="pp_v", bufs=2, space="PSUM"))
        pp_w = ctx.enter_context(tc.tile_pool(name="pp_w", bufs=2, space="PSUM"))
        pp_acc = ctx.enter_context(tc.tile_pool(name="pp_acc", bufs=4, space="PSUM"))
        pp_sm = pp_acc

        def load_const(dram, shape, dtype, tag):
            t = pc.tile(shape, dtype, tag=tag)
            nc.sync.dma_start(t[:], dram[:])
            return t

        # ---- constants ----
        e64 = load_const(dt_e64, [64, 64], F32R, "e64")
        e128 = load_const(dt_e128, [128, 128], F32R, "e128")
        vwT = [load_const(dt_vwT[i], [128, D], F32R, f"vwT{i}") for i in range(2)]
        kww = [load_const(dt_kww[i], [128, D], F32R, f"kww{i}") for i in range(2)]
        qwT = [load_const(dt_qwT[i], [128, D], F32R, f"qwT{i}") for i in range(2)]
        ewT = [load_const(dt_ewT[i], [128, 768], F32R, f"ewT{i}") for i in range(2)]
        l1T = [load_const(dt_l1T[i], [128, FF], BF16, f"l1T{i}") for i in range(2)]
        ruT = [load_const(dt_ruT[i], [128, D], BF16, f"ruT{i}") for i in range(2)]
        l2T = [load_const(dt_l2T[i], [128, D], BF16, f"l2T{i}") for i in range(8)]
        emb_tm = load_const(dt_emb, [NQ, D], F32, "emb_tm")
        ebT = [load_const(dt_embT[i], [128, NQ], F32R, f"ebT{i}") for i in range(2)]
        ones_f = load_const(dt_ones2, [128, 2], F32R, "ones_f")
        eps_t = pc.tile([128, 1], F32, name="eps_t", tag="eps_t")
        nc.vector.memset(eps_t[:], EPS)

        qbT = ebqT = ebvT = None
        if flags["qw_b"]:
            qbT = [load_const(dt_qb[i], [128, 1], F32, f"qbT{i}") for i in range(2)]
        if flags["embW_bq"]:
            ebqT = [load_const(dt_ebq[i], [128, 1], F32, f"ebqT{i}") for i in range(2)]
        if flags["embW_bv"]:
            ebvT = [load_const(dt_ebv[i], [128, 1], F32, f"ebvT{i}") for i in range(2)]
        vwrep = load_const(dt_vwrep, [128, D], F32, "vwrep") if flags["vw_b"] else None
        if flags["lin1_b"]:
            l1brow = load_const(dt_l1b, [1, FF], BF16, "l1brow")
            ones_row = load_const(dt_onesrow, [1, NQ], BF16, "ones_row")
        l2brep = load_const(dt_l2brep, [NQ, D], F32, "l2brep") if flags["lin2_b"] else None
        rubrep = load_const(dt_rubrep, [NQ, D], F32, "rubrep") if flags["resup_b"] else None
        n1g = load_const(dt_n1g, [NQ, D], F32, "n1g") if flags["n1g"] else None
        n1b = load_const(dt_n1b, [NQ, D], F32, "n1b") if flags["n1b"] else None
        png = load_const(dt_png, [NQ, D], F32, "png") if flags["png"] else None
        pnb = load_const(dt_pnb, [NQ, D], F32, "pnb") if flags["pnb"] else None

        def copy_ps(dst_ap, src_ap, eng):
            if eng == 0:
                nc.vector.tensor_copy(dst_ap, src_ap)
            else:
                nc.scalar.copy(dst_ap, src_ap)

        def layernorm(x_ap, out_ap, g, bvec):
            st = pnarrow.tile([NQ, 6], F32, name="ln_st", tag="ln_st")
            nc.vector.bn_stats(st[:], x_ap)
            mv = pnarrow.tile([NQ, 2], F32, name="ln_mv", tag="ln_mv")
            nc.vector.bn_aggr(mv[:], st[:])
            lnv = pnarrow.tile([NQ, 1], F32, name="ln_lnv", tag="ln_lnv")
            i_ln = nc.scalar.activation(lnv[:], mv[:, 1:2], AF.Ln, bias=eps_t[0:NQ, 0:1])
            rstd = pnarrow.tile([NQ, 1], F32, name="ln_rstd", tag="ln_rstd")
            nc.scalar.activation(rstd[:], lnv[:], AF.Exp, scale=-0.5)
            nmr = pnarrow.tile([NQ, 1], F32, name="ln_nmr", tag="ln_nmr")
            nc.vector.tensor_scalar(nmr[:], mv[:, 0:1], rstd[:, 0:1], -1.0,
                                    op0=OP.mult, op1=OP.mult)
            i_ap = nc.scalar.activation(out_ap, x_ap, AF.Identity,
                                 bias=nmr[:, 0:1], scale=rstd[:, 0:1])
            if g is not None:
                nc.vector.tensor_mul(out_ap, out_ap, g[:])
            if bvec is not None:
                nc.vector.tensor_add(out_ap, out_ap, bvec[:])
            return i_ln, i_ap

        # ============ phase 0: embedding self-attention (once per core) ====
        projs = [[], [], []]   # qeT, keT, veT feature-major [2][128, 64]
        pbias = [ebqT, None, ebvT]
        for pi in range(3):
            for mc in range(2):
                ps = pp_sm.tile([128, NQ], F32, name="ps_acc", tag="ps_acc")
                for cc in range(2):
                    nc.tensor.matmul(ps[:], f_r(ewT[cc][:, ts(2 * pi + mc, 128)]),
                                     f_r(ebT[cc][:]), start=cc == 0, stop=cc == 1)
                t = pc.tile([128, NQ], F32R, name=f"proj{pi}_{mc}", tag=f"proj{pi}_{mc}")
                if pbias[pi] is not None:
                    nc.scalar.activation(t[:], ps[:], AF.Identity,
                                         bias=pbias[pi][mc][:, 0:1])
                else:
                    nc.vector.tensor_copy(t[:], ps[:])
                projs[pi].append(t)
        qeT, keT, veT = projs

        ps_se = pp_sm.tile([NQ, NQ], F32, name="ps_acc", tag="ps_acc")
        for cc in range(2):
            nc.tensor.matmul(ps_se[:], f_r(keT[cc][:]), f_r(qeT[cc][:]),
                             start=cc == 0, stop=cc == 1)
        we = pc.tile([NQ, NQ], F32R, name="we", tag="we")
        nc.scalar.activation(we[:], ps_se[:], AF.Exp, scale=SCALE)
        ps_de = pp_sm.tile([NQ, 2], F32, name="ps_acc", tag="ps_acc")
        nc.tensor.matmul(ps_de[:], f_r(we[:]), f_r(ones_f[0:NQ, :]),
                         start=True, stop=True)

        ve_tm = pc.tile([NQ, D], F32R, name="ve_tm", tag="ve_tm")
        qe_tm = pc.tile([NQ, D], F32, name="qe_tm", tag="qe_tm")
        for cc in range(2):
            pt = pp_sm.tile([NQ, 128], F32R, name="ps_acc", tag="ps_acc")
            nc.tensor.transpose(pt[:], veT[cc][:], e128[:])
            nc.vector.tensor_copy(ve_tm[:, ts(cc, 128)], pt[:])
            pt2 = pp_sm.tile([NQ, 128], F32R, name="ps_acc", tag="ps_acc")
            nc.tensor.transpose(pt2[:], qeT[cc][:], e128[:])
            nc.vector.tensor_copy(qe_tm[:, ts(cc, 128)], pt2[:])

        ps_oe = pp_acc.tile([NQ, D], F32, name="ps_acc", tag="ps_acc")
        nc.tensor.matmul(ps_oe[:], f_r(we[:]), f_r(ve_tm[:]), start=True, stop=True)
        rde = pnarrow.tile([NQ, 1], F32, name="rde", tag="rde")
        nc.vector.reciprocal(rde[:], ps_de[:, 0:1])
        oe = pmisc.tile([NQ, D], F32, name="oe", tag="oe")
        nc.vector.tensor_scalar_mul(oe[:], ps_oe[:], rde[:, 0:1])
        nc.vector.tensor_add(oe[:], oe[:], qe_tm[:])
        ln_oe = pmisc.tile([NQ, D], F32, name="ln_oe", tag="ln_oe")
        layernorm(oe[:], ln_oe[:], n1g, n1b)
        embq2 = pc.tile([NQ, D], F32R, name="embq2", tag="embq2")
        nc.vector.tensor_add(embq2[:], ln_oe[:], emb_tm[:])

        embq2T = pc.tile([128, 128], F32R, name="embq2T", tag="embq2T")
        for cc in range(2):
            pt = pp_sm.tile([128, NQ], F32R, name="ps_acc", tag="ps_acc")
            nc.tensor.transpose(pt[:], embq2[:, ts(cc, 128)], e64[:])
            nc.vector.tensor_copy(embq2T[:, ts(cc, 64)], pt[:])

        qT = [pc.tile([128, NQ], F32R, name=f"qT{i}", tag=f"qT{i}") for i in range(2)]
        for mc in range(2):
            ps = pp_sm.tile([128, NQ], F32, name="ps_acc", tag="ps_acc")
            for kc in range(2):
                nc.tensor.matmul(ps[:], f_r(qwT[kc][:, ts(mc, 128)]),
                                 f_r(embq2T[:, ts(kc, 64)]),
                                 start=kc == 0, stop=kc == 1)
            if flags["qw_b"]:
                nc.scalar.activation(qT[mc][:], ps[:], AF.Identity,
                                     bias=qbT[mc][:, 0:1])
            else:
                nc.vector.tensor_copy(qT[mc][:], ps[:])
        q_tm = pc.tile([NQ, D], F32, name="q_tm", tag="q_tm")
        for mc in range(2):
            pt = pp_sm.tile([NQ, 128], F32R, name="ps_acc", tag="ps_acc")
            nc.tensor.transpose(pt[:], qT[mc][:], e128[:])
            nc.vector.tensor_copy(q_tm[:, ts(mc, 128)], pt[:])

        # kq[c, q] = kw^T @ q^T (contract d_out)
        kq = [pc.tile([128, NQ], im_dt, name=f"kq{i}", tag=f"kq{i}") for i in range(2)]
        for mc in range(2):
            ps = pp_sm.tile([128, NQ], F32, name="ps_acc", tag="ps_acc")
            for kc in range(2):
                nc.tensor.matmul(ps[:], f_r(kww[kc][:, ts(mc, 128)]),
                                 f_r(qT[kc][:]), start=kc == 0, stop=kc == 1)
            nc.vector.tensor_copy(kq[mc][:], ps[:])

        # epos = exp((posT x kq)/16)
        posT = [pim.tile([128, L], im_dt, name="imx", tag="imx") for _ in range(2)]
        for cc in range(2):
            nc.sync.dma_start(posT[cc][:], dt_pos[cc])
        epos = pc.tile([128, NLC * 64], at_dt, name="epos", tag="epos")
        for g in range(4):
            ps = pp_w.tile([128, 512], F32, name="ps_w", tag="ps_w")
            for j in range(8):
                lc = 8 * g + j
                for cc in range(2):
                    nc.tensor.matmul(ps[:, ts(j, 64)],
                                     im_r(posT[cc][:, ts(lc, 128)]),
                                     im_r(kq[cc][:]), start=cc == 0, stop=cc == 1)
            nc.scalar.activation(epos[:, ts(g, 512)], ps[:], AF.Exp, scale=SCALE)

        # ============ phase 1: attention per unit ==========================
        attn1s, ln1_applies, attn_lasts = [], [], []
        for u in range(UPC):
            imt = [pim.tile([128, L], im_dt, name="imx", tag="imx") for _ in range(2)]
            for cc in range(2):
                nc.sync.dma_start(imt[cc][:], dt_im[u, cc])

            # pooling: 8x8 block sums; bf16 pairwise halving (2x DVE mode).
            # fold yb halves within each yi row-block, then xb halves.
            pooled = []
            for cc in range(2):
                pa = pmisc.tile([128, 2048], BF16, name="poolA", tag="poolA")
                # imt free = y*64+x with y=yi*8+yb: view [yi=8, yb=8, x=64] -> fold yb
                v0 = imt[cc][:].rearrange("p (yi yb x) -> p yi yb x", yi=8, yb=8)
                nc.vector.tensor_add(pa[:].rearrange("p (yi yb x) -> p yi yb x", yi=8, yb=4),
                                     v0[:, :, 0:4, :], v0[:, :, 4:8, :])
                pb = pmisc.tile([128, 1024], BF16, name="poolB", tag="poolB")
                v1 = pa[:].rearrange("p (yi yb x) -> p yi yb x", yi=8, yb=4)
                nc.vector.tensor_add(pb[:].rearrange("p (yi yb x) -> p yi yb x", yi=8, yb=2),
                                     v1[:, :, 0:2, :], v1[:, :, 2:4, :])
                v2 = pb[:].rearrange("p (yi yb x) -> p yi yb x", yi=8, yb=2)
                nc.vector.tensor_add(pa[:, 0:512].rearrange("p (yi x) -> p yi x", yi=8),
                                     v2[:, :, 0, :], v2[:, :, 1, :])
                # now [yi=8, x=64]; fold xb: view [yi, xi=8, xb=8]
                v3 = pa[:, 0:512].rearrange("p (yi xi xb) -> p yi xi xb", yi=8, xi=8)
                nc.vector.tensor_add(pb[:, 0:256].rearrange("p (yi xi xb) -> p yi xi xb", yi=8, xi=8),
                                     v3[:, :, :, 0:4], v3[:, :, :, 4:8])
                v4 = pb[:, 0:256].rearrange("p (yi xi xb) -> p yi xi xb", yi=8, xi=8)
                nc.vector.tensor_add(pa[:, 0:128].rearrange("p (yi xi xb) -> p yi xi xb", yi=8, xi=8),
                                     v4[:, :, :, 0:2], v4[:, :, :, 2:4])
                v5 = pa[:, 0:128].rearrange("p (yi xi xb) -> p yi xi xb", yi=8, xi=8)
                p2r = pnarrow.tile([128, 64], BF16, name=f"pool2r_{cc}", tag=f"pool2r_{cc}")
                nc.vector.tensor_add(p2r[:].rearrange("p (yi xi) -> p yi xi", yi=8),
                                     v5[:, :, :, 0], v5[:, :, :, 1])
                pooled.append(p2r)
            ps_res = pp_acc.tile([NQ, D], F32, name="ps_acc", tag="ps_acc")
            for cc in range(2):
                nc.tensor.matmul(ps_res[:], f_r(pooled[cc][:]), f_r(ruT[cc][:]),
                                 start=cc == 0, stop=cc == 1)
            res_sb = pmisc.tile([NQ, D], F32, name="res_sb", tag="res_sb")
            if flags["resup_b"]:
                nc.vector.tensor_add(res_sb[:], ps_res[:], rubrep[:])
            else:
                nc.vector.tensor_copy(res_sb[:], ps_res[:])

            # transposed image tiles [L-chunk partitions, (c | ones)] for the
            # value-side contraction G = sum_L w * im_tm
            im_tm = pvsb.tile([128, NLC, 258], at_dt, name="im_tm", tag="im_tm")
            nc.sync.dma_start(im_tm[:], dt_imtm[u])

            # scores (im part) -> exp -> * epos
            expw_raw = pexp.tile([128, NLC * 64], at_dt, name="expw_raw", tag="expw_raw")
            for g in range(4):
                ps = pp_w.tile([128, 512], F32, name="ps_w", tag="ps_w")
                for j in range(8):
                    lc = 8 * g + j
                    for cc in range(2):
                        nc.tensor.matmul(ps[:, ts(j, 64)],
                                         im_r(imt[cc][:, ts(lc, 128)]),
                                         im_r(kq[cc][:]), start=cc == 0, stop=cc == 1)
                nc.scalar.activation(expw_raw[:, ts(g, 512)], ps[:],
                                     AF.Exp, scale=SCALE)
            expw = pexp.tile([128, NLC * 64], at_dt, name="expw", tag="expw")
            nc.vector.tensor_mul(expw[:], expw_raw[:], epos[:])

            # G = sum_L w * im_tm (ones col -> denominator), then out = (G/den) @ vwT
            ps_g = pp_acc.tile([NQ, 258], F32, name="ps_acc", tag="ps_acc")
            for j in range(NLC):
                nc.tensor.matmul(ps_g[:], at_r(expw[:, ts(j, 64)]),
                                 at_r(im_tm[:, j, :]),
                                 start=j == 0, stop=j == NLC - 1)

            rden = pnarrow.tile([NQ, 1], F32, name="rden", tag="rden")
            nc.vector.reciprocal(rden[:], ps_g[:, 256:257])
            g_sb = pmisc.tile([NQ, D], F32R, name="g_sb", tag="g_sb")
            nc.vector.tensor_scalar_mul(g_sb[:], ps_g[:, 0:256], rden[:, 0:1])
            gT = pmisc.tile([128, 128], F32R, name="gT", tag="gT")
            for cc in range(2):
                ptg = pp_acc.tile([128, NQ], F32R, name="ps_acc", tag="ps_acc")
                nc.tensor.transpose(ptg[:], g_sb[:, ts(cc, 128)], e64[:])
                nc.vector.tensor_copy(gT[:, ts(cc, 64)], ptg[:])
            ps_o = pp_acc.tile([NQ, D], F32, name="ps_acc", tag="ps_acc")
            i_ao = None
            for cc in range(2):
                i_ao = nc.tensor.matmul(ps_o[:], gT[:, ts(cc, 64)], vwT[cc][:],
                                        start=cc == 0, stop=cc == 1)
            o_sb = pmisc.tile([NQ, D], F32, name="o_sb", tag="o_sb")
            nc.vector.tensor_add(o_sb[:], ps_o[:], q_tm[:])
            if flags["vw_b"]:
                nc.vector.tensor_add(o_sb[:], o_sb[:], vwrep[0:NQ, :])
            ln1 = pmisc.tile([NQ, D], F32, name="ln1", tag="ln1")
            _, i_ln1 = layernorm(o_sb[:], ln1[:], n1g, n1b)
            ln1_applies.append(i_ln1)
            attn_lasts.append(i_ao)
            a1 = pa1.tile([NQ, D], F32R, name="attn1", tag="attn1")
            nc.vector.tensor_add(a1[:], ln1[:], res_sb[:])
            attn1s.append(a1)

        # ============ phase 2a: transpose + lin1 + gelu (batched) ==========
        attn1Ts, hTs, gelus, lin1_firsts = [], [], [], []
        for u in range(UPC):
            a1 = attn1s[u]
            a1T = pa1t.tile([128, 128], BF16, name="attn1T", tag="attn1T")
            for cc in range(2):
                pt = pp_sm.tile([128, NQ], F32R, name="ps_acc", tag="ps_acc")
                nc.tensor.transpose(pt[:], a1[:, ts(cc, 128)], e64[:])
                nc.vector.tensor_copy(a1T[:, ts(cc, 64)], pt[:])
            attn1Ts.append(a1T)

            ps_h = pp_v.tile([128, 512], F32, name="ps_v", tag="ps_v")
            i_first = None
            for fc in range(8):
                for cc in range(2):
                    last = (cc == 1) and not flags["lin1_b"]
                    i_mm = nc.tensor.matmul(ps_h[:, ts(fc, 64)],
                                            l1T[cc][:, ts(fc, 128)],
                                            a1T[:, ts(cc, 64)],
                                            start=cc == 0, stop=last)
                    if i_first is None:
                        i_first = i_mm
                if flags["lin1_b"]:
                    nc.tensor.matmul(ps_h[:, ts(fc, 64)],
                                     l1brow[0:1, ts(fc, 128)],
                                     ones_row[:], start=False, stop=True)
            lin1_firsts.append(i_first)
            hT = pht.tile([128, 512], BF16, name="hT", tag="hT")
            gelus.append(nc.scalar.activation(hT[:], ps_h[:], AF.Gelu))
            hTs.append(hT)

        # ============ phase 2b: lin2 + residual + LN2 + store ==============
        ln2_lns = []
        for u in range(UPC):
            ps_o2 = pp_acc.tile([NQ, D], F32, name="ps_acc", tag="ps_acc")
            for fc in range(8):
                nc.tensor.matmul(ps_o2[:], hTs[u][:, ts(fc, 64)],
                                 l2T[fc][:], start=fc == 0, stop=fc == 7)
            o2 = pmisc.tile([NQ, D], F32, name="o2", tag="o2")
            nc.vector.tensor_add(o2[:], ps_o2[:], attn1s[u][:].bitcast(F32))
            if flags["lin2_b"]:
                nc.vector.tensor_add(o2[:], o2[:], l2brep[:])
            out_sb = pmisc.tile([NQ, D], F32, name="out_sb", tag="out_sb")
            i_ln2, _ = layernorm(o2[:], out_sb[:], png, pnb)
            ln2_lns.append(i_ln2)
            nc.scalar.dma_start(dt_out[u], out_sb[:])

        # ---- phase-ordering deps: keep ACT table sets batched ----
        for u in range(UPC):
            for w in range(UPC):
                add_dep_helper(lin1_firsts[u].ins, attn_lasts[w].ins,
                               sync=False, reason="phase2a PE after phase1")
                add_dep_helper(gelus[u].ins, ln1_applies[w].ins,
                               sync=False, reason="batch gelu table set")
                add_dep_helper(ln2_lns[u].ins, gelus[w].ins,
                               sync=False, reason="batch nlx table set")

    nc.compile()
    return nc


def _host_prep(inputs):
    im = np.asarray(inputs["im"], np.float32)
    emb = np.asarray(inputs["emb"], np.float32)
    g = lambda k: np.asarray(inputs[k], np.float32)
    im_np_dt = np.dtype(mybir.dt.np(BF16)) if IM_BF16 else np.float32

    flags = {
        "qw_b": bool(np.any(g("qw_b"))),
        "vw_b": bool(np.any(g("vw_b"))),
        "embW_bq": bool(np.any(g("embW_b")[0:256])),
        "embW_bv": bool(np.any(g("embW_b")[512:768])),
        "lin1_b": bool(np.any(g("lin1_b"))),
        "lin2_b": bool(np.any(g("lin2_b"))),
        "resup_b": bool(np.any(g("resup_b"))),
        "n1g": bool(np.any(g("norm1_g") != 1.0)),
        "n1b": bool(np.any(g("norm1_b"))),
        "png": bool(np.any(g("post_norm_g") != 1.0)),
        "pnb": bool(np.any(g("post_norm_b"))),
    }

    pos = _pos_sine_np()                                   # [L, D]
    shared = {
        "posT": np.ascontiguousarray(pos.T).reshape(2, 128, L).astype(im_np_dt),
        "vwT": np.ascontiguousarray(g("vw_w").T).reshape(2, 128, D),
        "kww": np.ascontiguousarray(g("kw_w")).reshape(2, 128, D),
        "qwT": np.ascontiguousarray(g("qw_w").T).reshape(2, 128, D),
        "embWT": np.ascontiguousarray(g("embW_w").T).reshape(2, 128, 768),
        "lin1T": np.ascontiguousarray(g("lin1_w").T).reshape(2, 128, FF).astype(np.dtype(mybir.dt.np(BF16))),
        "lin2T": np.ascontiguousarray(g("lin2_w").T).reshape(8, 128, D).astype(np.dtype(mybir.dt.np(BF16))),
        "resupT": np.ascontiguousarray((g("resup_w") / 64.0).T).reshape(2, 128, D).astype(np.dtype(mybir.dt.np(BF16))),
        "eye64": np.eye(64, dtype=np.float32),
        "eye128": np.eye(128, dtype=np.float32),
        "qw_bT": g("qw_b").reshape(2, 128, 1),
        "embW_bqT": g("embW_b")[0:256].reshape(2, 128, 1),
        "embW_bvT": g("embW_b")[512:768].reshape(2, 128, 1),
        "vw_rep": np.ascontiguousarray(np.tile(g("vw_b"), (128, 1))),
        "lin1b_row": g("lin1_b").reshape(1, FF).astype(np.dtype(mybir.dt.np(BF16))),
        "lin2b_rep": np.ascontiguousarray(np.tile(g("lin2_b"), (NQ, 1))),
        "resupb_rep": np.ascontiguousarray(np.tile(g("resup_b"), (NQ, 1))),
        "n1g_rep": np.ascontiguousarray(np.tile(g("norm1_g"), (NQ, 1))),
        "n1b_rep": np.ascontiguousarray(np.tile(g("norm1_b"), (NQ, 1))),
        "png_rep": np.ascontiguousarray(np.tile(g("post_norm_g"), (NQ, 1))),
        "pnb_rep": np.ascontiguousarray(np.tile(g("post_norm_b"), (NQ, 1))),
        "ones_col2": np.ones((128, 2), np.float32),
        "ones_rowq": np.ones((1, NQ), np.dtype(mybir.dt.np(BF16))),
    }

    in_maps = []
    for core in range(NCORES):
        b, sh = core // 2, core % 2
        # im[b]: [c, y, x] -> tiles [16, c, 64*64], keep this core's 8
        A = im[b].reshape(D, 4, 64, 4, 64).transpose(1, 3, 0, 2, 4)
        A = A.reshape(16, D, L)[sh * UPC:(sh + 1) * UPC]
        m = dict(shared)
        m["imt"] = np.ascontiguousarray(A).reshape(UPC, 2, 128, L).astype(im_np_dt)
        # [u, p, lc, c|ones]: value-side transposed tiles with ones columns
        Bm = A.reshape(UPC, D, 32, 128).transpose(0, 3, 2, 1).astype(im_np_dt)
        tm = np.empty((UPC, 128, 32, 258), im_np_dt)
        tm[:, :, :, 0:256] = Bm
        tm[:, :, :, 256:258] = np.asarray(1.0, im_np_dt)
        m["imtm"] = tm
        m["emb_b"] = np.ascontiguousarray(emb[b])
        m["embT"] = np.ascontiguousarray(emb[b].T).reshape(2, 128, NQ)
        in_maps.append(m)
    return flags, in_maps


def kernel(**inputs):
    global LAST_EXEC_NS, LAST_RESULTS
    flags, in_maps = _host_prep(inputs)
    nc = build_nc(flags)
    res = run_bass_kernel_spmd(nc, in_maps, list(range(NCORES)), trace=TRACE)
    LAST_EXEC_NS = res.exec_time_ns
    LAST_RESULTS = res
    out = np.empty((B, 16, NQ, D), np.float32)
    for core in range(NCORES):
        b, sh = core // 2, core % 2
        out[b, sh * UPC:(sh + 1) * UPC] = res.results[core]["out"]
    return out.reshape(B, 16 * NQ, D)

